# revision 27
# baseline (speedup 1.0000x reference)
"""Trainium2 Bass kernel for per-head Llama GQA attention.

Model: H=16 q heads, HKV=4 kv heads, head_dim=128, L=2048, D=2048, B=1.
Per-head hidden streams and per-head outputs (no cross-head reduction), so
tensor-parallel over heads is embarrassingly parallel: core c owns q heads
{2c, 2c+1} and their kv head c//2.  No collectives.

The causal structure lets the whole kernel pipeline at 512-column (chunk)
granularity: attention chunk c needs K/V columns < (c+1)*512 only, so the
emission order is  [V,Q,K proj + rope of chunk c] -> attn(head0, c) ->
outproj(head0, c-1)  and the input streams never have to finish before
compute starts.  Hidden streams are host-packed chunk-major
([NCH, dtg, 128, 4, 512]) so chunk streaming stays DMA-contiguous.

Queue model (measured): gpsimd SWDGE ~266 GB/s, sync/scalar HWDGE ~100
GB/s each, ~358 GB/s per-core aggregate.  gpsimd carries xv + xq + the
fattest consts in consumption order; xk is split between sync (first
d-half) and scalar (second d-half); output stores rotate over all three
queues (16 MB of stores on a single 100 GB/s queue would otherwise be a
160 us serial tail).

Other notes:
  - per-chunk projections accumulate in single-bank (128,512) PSUM tiles;
    scores use (128,1024) pair tiles; PSUM = 4x small + 2x big = 8 banks.
  - RoPE in (HD, L) layout: rotate_half is a 128x128 signed permutation
    matmul; cos/sin are host-precomputed fp16 (HD, L) tables with
    1/sqrt(HD) folded into the q tables.
  - exp on ACT with a -2.0 bias (numerator and denominator scale
    together; keeps fp16 row-sum accumulation far from overflow).
  - softmax denominator: DVE accumulates exp tiles pairwise into an fp16
    (j_local, l) partial-sum tile; 4 single-column matmuls per chunk (acc
    slice stationary, ones column moving) give the (l,1) denominators in
    PSUM; the reciprocal is folded into the out-projection PSUM->SBUF
    copies as a per-partition scale.
"""

import os
import sys

sys.path.insert(0, "/opt/trn_rl_repo")

import numpy as np

import concourse.bass as bass
import concourse.tile as tile
from concourse import bacc, mybir
from concourse.bass_utils import run_bass_kernel_spmd

H, HKV, D, HD, L = 16, 4, 2048, 128, 2048
THETA = 10000.0
NC = 8
HPC = H // NC  # q heads per core (2)
NDT = D // 128  # d-tiles (16)
NDG = 4  # d-tile groups per stream transfer
NLT = L // 128  # l/j tiles (16)
NCH = L // 512  # 512-wide chunks (4)
F16 = mybir.dt.float16
F32 = mybir.dt.float32
EXP = mybir.ActivationFunctionType.Exp
CPY = mybir.ActivationFunctionType.Copy
EXP_BIAS = -2.0  # exp(s-2): cancels in softmax, keeps fp16 sums small

last_exec_time_ns = None
last_mean_exec_time_ns = None

_programs = {}


def _build_program(causal: bool) -> bass.Bass:
    # Bacc (not plain Bass): its compile() runs the wait-splitting passes
    # (generate_event_semaphores) that walrus requires — pseudo-DMA
    # instructions may carry at most one embedded sync wait.
    nc = bacc.Bacc(None, target_bir_lowering=False)

    # hidden streams chunk-major: [chunk, dtg, 128, di, 512]
    xq = nc.dram_tensor("xq", [HPC, NCH, NDG, 128, NDT // NDG, 512], F16,
                        kind="ExternalInput")
    xk = nc.dram_tensor("xk", [NCH, NDG, 128, NDT // NDG, 512], F16,
                        kind="ExternalInput")
    xv = nc.dram_tensor("xv", [NCH, NDG, 128, NDT // NDG, 512], F16,
                        kind="ExternalInput")
    wq = nc.dram_tensor("wq", [128, HPC, NDT, 128], F16, kind="ExternalInput")
    wk = nc.dram_tensor("wk", [128, NDT, 128], F16, kind="ExternalInput")
    wv = nc.dram_tensor("wv", [128, NDT, 128], F16, kind="ExternalInput")
    wo = nc.dram_tensor("wo", [128, HPC, D], F16, kind="ExternalInput")
    # only the q-scaled rope tables ship; k tables are derived on-device
    cosq = nc.dram_tensor("cosq", [128, L], F16, kind="ExternalInput")
    sinq = nc.dram_tensor("sinq", [128, L], F16, kind="ExternalInput")
    # misc fp16 constants: [:, :128] rotate-half perm (lhsT), [:, 128] ones
    # col, [:, 129] exp-bias col
    misc = nc.dram_tensor("misc", [128, 132], F16, kind="ExternalInput")
    if causal:
        mask4 = nc.dram_tensor("mask4", [128, 2, 1024], F16, kind="ExternalInput")
    else:
        maskg = nc.dram_tensor("maskg", [128, NLT // 2, NCH, 1024], F32,
                               kind="ExternalInput")
    out = nc.dram_tensor("out", [HPC, NLT, 128, D], F16, kind="ExternalOutput")

    with tile.TileContext(nc) as tc:
        with (
            tc.tile_pool(name="const", bufs=1) as constp,
            tc.tile_pool(name="xvp", bufs=6) as xvp,
            tc.tile_pool(name="xkp", bufs=8) as xkp,
            tc.tile_pool(name="xqp", bufs=10) as xqp,
            tc.tile_pool(name="persist", bufs=1) as persist,
            tc.tile_pool(name="probs", bufs=8) as probsp,
            tc.tile_pool(name="accs", bufs=3) as accp,
            tc.tile_pool(name="small", bufs=3) as small,
            tc.tile_pool(name="att16", bufs=3) as att16p,
            tc.tile_pool(name="outs", bufs=4) as outsp,
            tc.tile_pool(name="recs", bufs=8) as recs,
            # PSUM: 8 banks = 4x (128,512) small + 2x (128,1024) big
            tc.tile_pool(name="psml", bufs=4, space="PSUM") as psml,
            tc.tile_pool(name="pbig", bufs=2, space="PSUM") as pbig,
        ):
            # ---- const tiles ----
            wv_sb = constp.tile([128, NDT, 128], F16, tag="wv")
            wk_sb = constp.tile([128, NDT, 128], F16, tag="wk")
            w_all = constp.tile([128, HPC, NDT, 128], F16, tag="wq")
            wo_sb = constp.tile([128, HPC, D], F16, tag="wo")
            misc_sb = constp.tile([128, 132], F16, tag="misc")
            cosk_sb = constp.tile([128, L], F16, tag="cosk")
            sink_sb = constp.tile([128, L], F16, tag="sink")
            cosq_sb = constp.tile([128, L], F16, tag="cosq")
            sinq_sb = constp.tile([128, L], F16, tag="sinq")
            if causal:
                mask_sb = constp.tile([128, 2, 1024], F16, tag="mask")
            perm = misc_sb[:, 0:128]
            ones_col = misc_sb[:, 128:129]
            ebias_col = misc_sb[:, 129:130]

            # const DMAs, interleaved with the streams by deadline:
            # gpsimd: wv, wq, cosq/sinq, mask (fast queue, early deadlines)
            # sync:   misc, cosk, sink  (ahead of its xk share)
            # scalar: wk
            nc.sync.dma_start(out=misc_sb[:], in_=misc[:])
            nc.scalar.dma_start(out=wk_sb[:], in_=wk[:])

            # persistent per-core activations
            krot = persist.tile([128, L], F16, tag="krot")
            v16 = persist.tile([128, L], F16, tag="v16")
            qrot = [
                persist.tile([128, L], F16, tag=f"qrot{i}", name=f"qrot{i}")
                for i in range(HPC)
            ]

            store_rr = [0]

            def emit_store(dst, src):
                # rotate output stores g, s, a, g: gpsimd takes half
                r = store_rr[0] % 4
                store_rr[0] += 1
                eng = (nc.gpsimd, nc.sync, nc.scalar, nc.gpsimd)[r]
                eng.dma_start(out=dst, in_=src)

            # ---- chunk-granular stream loads.  One (128, 4, 512) 512 KB
            # transfer per (chunk, d-group); slab [:, di, :] is the (128 d,
            # 512 l) tile of d-tile dt = 4*g + di. ----
            def load_xv(c):
                # chunk 0 rides the HWDGE queues for d-groups 0/1 (the
                # SWDGE queue's first transfer lands ~13us late); each wv
                # quarter is interleaved ahead of its xv transfer
                ts = []
                for g in range(NDG):
                    if c == 0:
                        eng = (nc.sync, nc.scalar, nc.gpsimd, nc.gpsimd)[g]
                        eng.dma_start(
                            out=wv_sb[:, g * 4 : (g + 1) * 4, :],
                            in_=wv[:, g * 4 : (g + 1) * 4, :],
                        )
                    else:
                        eng = nc.gpsimd
                    t = xvp.tile([128, NDT // NDG, 512], F16, tag="xv",
                                 name=f"xv{c}_{g}")
                    eng.dma_start(out=t[:], in_=xv[c, g])
                    ts.append(t)
                return ts

            def load_xk(c):
                # first d-half on sync, second on scalar
                ts = []
                for g in range(NDG):
                    t = xkp.tile([128, NDT // NDG, 512], F16, tag="xk",
                                 name=f"xk{c}_{g}")
                    eng = nc.sync if g < 2 else nc.scalar
                    eng.dma_start(out=t[:], in_=xk[c, g])
                    ts.append(t)
                return ts

            def load_xq(i, c):
                ts = []
                for g in range(NDG):
                    t = xqp.tile([128, NDT // NDG, 512], F16, tag="xq",
                                 name=f"xq{i}_{c}_{g}")
                    nc.gpsimd.dma_start(out=t[:], in_=xq[i, c, g])
                    ts.append(t)
                return ts

            def v_chunk(c, xvt_groups):
                # v chunk c: l-tiles 4c..4c+3 in (l_local, hd) layout
                pv = psml.tile([128, 512], F32, tag="ps", name=f"pv{c}")
                for dt in range(NDT):
                    g, di = dt // NDG, dt % NDG
                    xsl = xvt_groups[g][:, di, :]  # (128 d, 512 l)
                    for m in range(4):
                        nc.tensor.matmul(
                            pv[:, m * 128 : (m + 1) * 128],
                            xsl[:, m * 128 : (m + 1) * 128],
                            wv_sb[:, dt, :],
                            start=(dt == 0 and m == 0),
                            stop=(dt == NDT - 1 and m == 3),
                            skip_group_check=True,
                        )
                nc.vector.tensor_copy(
                    out=v16[:, c * 512 : (c + 1) * 512], in_=pv[:]
                )

            def kq_chunk(c, w_sb, x_groups, cos_sb, sin_sb, dst):
                # (hd, l) projection of one 512-chunk + rope
                pp = psml.tile([128, 512], F32, tag="ps", name=f"pp{c}")
                for dt in range(NDT):
                    g, di = dt // NDG, dt % NDG
                    nc.tensor.matmul(
                        pp[:],
                        w_sb[:, dt, :],
                        x_groups[g][:, di, :],
                        start=(dt == 0),
                        stop=(dt == NDT - 1),
                    )
                csl = slice(c * 512, (c + 1) * 512)
                u16 = small.tile([128, 512], F16, tag="u16")
                nc.scalar.copy(out=u16[:], in_=pp[:])
                rh = psml.tile([128, 512], F32, tag="ps", name=f"rh{c}")
                nc.tensor.matmul(rh[:], perm, u16[:])
                tmp = small.tile([128, 512], F16, tag="ropetmp")
                nc.vector.tensor_mul(out=tmp[:], in0=u16[:], in1=cos_sb[:, csl])
                nc.vector.tensor_mul(out=dst[:, csl], in0=rh[:], in1=sin_sb[:, csl])
                nc.vector.tensor_add(out=dst[:, csl], in0=dst[:, csl], in1=tmp[:])

            # ---- attention chunk (same dataflow as before) ----
            def attn_phase(i, c):
                njt = 4 * c + 4 if causal else NLT
                pattn = psml.tile([128, 512], F32, tag="ps", name="pattn")
                acc = accp.tile([128, 512], F16, tag="acc")
                qsl = qrot[i][:, c * 512 : (c + 1) * 512]
                for jp in range(njt // 2):
                    jt0 = 2 * jp
                    sp = pbig.tile([128, 1024], F32, tag="pbig")
                    nc.tensor.matmul(
                        sp[:, 0:512], krot[:, jt0 * 128 : (jt0 + 1) * 128], qsl
                    )
                    nc.tensor.matmul(
                        sp[:, 512:1024],
                        krot[:, (jt0 + 1) * 128 : (jt0 + 2) * 128],
                        qsl,
                    )
                    if causal:
                        if jt0 >= 4 * c:
                            nc.vector.tensor_add(
                                out=sp[:],
                                in0=sp[:],
                                in1=mask_sb[:, (jt0 - 4 * c) // 2, :],
                            )
                    else:
                        mg = small.tile([128, 1024], F32, tag="maskg")
                        nc.gpsimd.dma_start(out=mg[:], in_=maskg[:, jp, c, :])
                        nc.vector.tensor_add(out=sp[:], in0=sp[:], in1=mg[:])
                    pe = probsp.tile([128, 1024], F16, tag="probs")
                    nc.scalar.activation(out=pe[:], in_=sp[:], func=EXP, bias=ebias_col)
                    last = jp == njt // 2 - 1
                    nc.tensor.matmul(
                        pattn[:],
                        v16[:, jt0 * 128 : (jt0 + 1) * 128],
                        pe[:, 0:512],
                        start=(jp == 0), stop=False,
                    )
                    nc.tensor.matmul(
                        pattn[:],
                        v16[:, (jt0 + 1) * 128 : (jt0 + 2) * 128],
                        pe[:, 512:1024],
                        start=False, stop=last,
                    )
                    # fp16 row-sum partials on DVE
                    if jp == 0:
                        nc.vector.tensor_add(
                            out=acc[:], in0=pe[:, 0:512], in1=pe[:, 512:1024]
                        )
                    else:
                        nc.vector.tensor_add(out=acc[:], in0=acc[:], in1=pe[:, 0:512])
                        nc.vector.tensor_add(out=acc[:], in0=acc[:], in1=pe[:, 512:1024])
                attn16 = att16p.tile([128, 512], F16, tag="attn16")
                nc.scalar.copy(out=attn16[:], in_=pattn[:])
                return acc, attn16

            def outproj_phase(i, c, acc, attn16):
                # denominators: acc slice stationary, ones column moving ->
                # (l_local, 1) column sums directly in PSUM
                pdg = psml.tile([128, 4], F32, tag="ps", name="pdg")
                for ls in range(4):
                    nc.tensor.matmul(
                        pdg[:, ls : ls + 1],
                        acc[:, ls * 128 : (ls + 1) * 128],
                        ones_col,
                    )
                recip = recs.tile([128, 4], F32, tag="recip")
                nc.vector.reciprocal(out=recip[:], in_=pdg[:])
                for ls in range(4):
                    lt = 4 * c + ls
                    a_sl = attn16[:, ls * 128 : (ls + 1) * 128]
                    r_sl = recip[:, ls : ls + 1]
                    ost = outsp.tile([128, D], F16, tag="ost")
                    for dp in range(4):
                        po = psml.tile([128, 512], F32, tag="ps", name="po")
                        nc.tensor.matmul(
                            po[:],
                            a_sl,
                            wo_sb[:, i, dp * 512 : (dp + 1) * 512],
                        )
                        hsl = slice(dp * 512, (dp + 1) * 512)
                        if dp % 2 == 0:
                            nc.vector.tensor_scalar_mul(
                                out=ost[:, hsl], in0=po[:], scalar1=r_sl
                            )
                        else:
                            nc.scalar.activation(
                                out=ost[:, hsl], in_=po[:], func=CPY, scale=r_sl
                            )
                    emit_store(out[i, lt], ost[:])

            # ================= emission =================
            def derive_k_tables():
                # k tables = q tables * sqrt(HD) (undo the folded 1/sqrt(HD))
                s = float(np.sqrt(HD))
                nc.vector.tensor_scalar_mul(
                    out=cosk_sb[:], in0=cosq_sb[:], scalar1=s
                )
                nc.vector.tensor_scalar_mul(
                    out=sink_sb[:], in0=sinq_sb[:], scalar1=s
                )

            if causal:
                xq1_tiles = [None] * NCH
                pending = None
                for c in range(NCH):
                    xv_groups = load_xv(c)
                    if c == 0:
                        # sync: misc, wv_g0, xv_c0_g0, cosq, sinq, xka...
                        # scalar: wk, wv_g1, xv_c0_g1, wq(head0), xkb...
                        nc.sync.dma_start(out=cosq_sb[:], in_=cosq[:])
                        nc.sync.dma_start(out=sinq_sb[:], in_=sinq[:])
                        derive_k_tables()
                        nc.scalar.dma_start(out=w_all[:, 0], in_=wq[:, 0])
                    xq_groups = load_xq(0, c)
                    if c == 0:
                        nc.gpsimd.dma_start(out=mask_sb[:], in_=mask4[:])
                    xk_groups = load_xk(c)
                    if c == 1:
                        nc.gpsimd.dma_start(out=wo_sb[:], in_=wo[:])
                    if c >= 2:
                        if c == 2:
                            nc.gpsimd.dma_start(out=w_all[:, 1], in_=wq[:, 1])
                        xq1_tiles[c - 2] = load_xq(1, c - 2)

                    v_chunk(c, xv_groups)
                    kq_chunk(c, w_all[:, 0], xq_groups, cosq_sb, sinq_sb, qrot[0])
                    kq_chunk(c, wk_sb, xk_groups, cosk_sb, sink_sb, krot)
                    cur = attn_phase(0, c)
                    if pending is not None:
                        outproj_phase(*pending)
                    pending = (0, c, *cur)
                # head 1, software-pipelined one chunk deeper: the q
                # projection of chunk c+1 is emitted before attention of
                # chunk c, so PE never waits on the rope DVE chain
                xq1_tiles[2] = load_xq(1, 2)
                kq_chunk(0, w_all[:, 1], xq1_tiles[0], cosq_sb, sinq_sb, qrot[1])
                for c in range(NCH):
                    if c + 3 < NCH:
                        xq1_tiles[c + 3] = load_xq(1, c + 3)
                    if c + 1 < NCH:
                        kq_chunk(c + 1, w_all[:, 1], xq1_tiles[c + 1],
                                 cosq_sb, sinq_sb, qrot[1])
                    cur = attn_phase(1, c)
                    outproj_phase(*pending)
                    pending = (1, c, *cur)
                outproj_phase(*pending)
            else:
                # generic mask: attention chunk c needs all K/V chunks, so
                # run all projections first, then the attention loop
                nc.gpsimd.dma_start(out=w_all[:], in_=wq[:])
                nc.sync.dma_start(out=cosq_sb[:], in_=cosq[:])
                nc.sync.dma_start(out=sinq_sb[:], in_=sinq[:])
                derive_k_tables()
                nc.gpsimd.dma_start(out=wo_sb[:], in_=wo[:])
                for c in range(NCH):
                    xv_groups = load_xv(c)
                    xk_groups = load_xk(c)
                    v_chunk(c, xv_groups)
                    kq_chunk(c, wk_sb, xk_groups, cosk_sb, sink_sb, krot)
                for i in range(HPC):
                    for c in range(NCH):
                        xq_groups = load_xq(i, c)
                        kq_chunk(c, w_all[:, i], xq_groups, cosq_sb, sinq_sb,
                                 qrot[i])
                pending = None
                for i in range(HPC):
                    for c in range(NCH):
                        cur = attn_phase(i, c)
                        if pending is not None:
                            outproj_phase(*pending)
                        pending = (i, c, *cur)
                outproj_phase(*pending)
    nc.compile()
    return nc


def _get_program(causal: bool) -> bass.Bass:
    if causal not in _programs:
        _programs[causal] = _build_program(causal)
    return _programs[causal]


def _rope_tables(position_ids: np.ndarray):
    pos = position_ids.reshape(-1).astype(np.float32)  # (L,)
    inv_freq = (
        1.0 / (THETA ** (np.arange(0, HD, 2, dtype=np.float32) / HD))
    ).astype(np.float32)
    freqs = pos[:, None] * inv_freq[None, :]  # (L, HD/2)
    emb = np.concatenate([freqs, freqs], axis=1)  # (L, HD)
    cos = np.cos(emb).T.astype(np.float32).copy()  # (HD, L)
    sin = np.sin(emb).T.astype(np.float32).copy()
    return cos, sin


def _chunk_tiles(x):
    """(L, D) fp32 -> [NCH, NDG, 128, NDT//NDG, 512] fp16: transfer (c, g)
    is (128, 4, 512) with partition p = d-row within d-tile 4g+di, slab di
    holding l-columns [512c, 512c+512)."""
    xt = x.T.astype(np.float16).reshape(NDT, 128, L)  # (dt, p, l)
    a = xt.reshape(NDG, NDT // NDG, 128, NCH, 512)  # (g, di, p, c, l')
    return np.ascontiguousarray(a.transpose(3, 0, 2, 1, 4))


def kernel(
    q_hidden, k_hidden, v_hidden, wq, wk, wv, wo, attention_mask, position_ids
):
    global last_exec_time_ns, last_mean_exec_time_ns
    q_hidden = np.asarray(q_hidden)
    k_hidden = np.asarray(k_hidden)
    v_hidden = np.asarray(v_hidden)
    wq = np.asarray(wq, dtype=np.float32)
    wk = np.asarray(wk, dtype=np.float32)
    wv = np.asarray(wv, dtype=np.float32)
    wo = np.asarray(wo, dtype=np.float32)
    attention_mask = np.asarray(attention_mask, dtype=np.float32)
    position_ids = np.asarray(position_ids)

    mask2d = attention_mask.reshape(L, L)
    causal_ref = np.where(
        np.tril(np.ones((L, L), dtype=bool)), np.float32(0.0), np.float32(-1e9)
    )
    causal = bool(np.array_equal(mask2d, causal_ref))

    cos, sin = _rope_tables(position_ids)
    scale = np.float32(1.0 / np.sqrt(HD))
    cosq_h = (cos * scale).astype(np.float16)
    sinq_h = (sin * scale).astype(np.float16)

    # diagonal-band causal mask tiles: allowed iff j_local + 128*r <= l_local,
    # packed as j-tile pairs: pair p holds r=2p | r=2p+1 side by side.
    # -30000 (fp16-exact) underflows exp() to 0 in fp32 just like -1e9.
    jj = np.arange(128, dtype=np.int32)[:, None]
    ll = np.arange(512, dtype=np.int32)[None, :]
    _mr = [
        np.where(jj + 128 * r <= ll, np.float16(0.0), np.float16(-30000.0))
        .astype(np.float16)
        for r in range(4)
    ]
    mask4_h = np.stack(
        [np.concatenate([_mr[0], _mr[1]], axis=1),
         np.concatenate([_mr[2], _mr[3]], axis=1)],
        axis=1,
    )  # (128, 2, 1024) fp16

    misc_h = np.zeros((128, 132), dtype=np.float16)
    # rotate-half: rh = P @ q with P[i, i+64] = -1 (i<64), P[i, i-64] = +1;
    # stored as lhsT = P^T
    for a in range(64):
        misc_h[a, a + 64] = np.float16(1.0)  # P^T[a, a+64] = P[a+64, a] = +1
        misc_h[a + 64, a] = np.float16(-1.0)  # P^T[a+64, a] = P[a, a+64] = -1
    misc_h[:, 128] = np.float16(1.0)  # ones column
    misc_h[:, 129] = np.float16(EXP_BIAS)  # exp bias column

    wq_r = wq.reshape(H, HD, D)
    wk_r = wk.reshape(HKV, HD, D)
    wv_r = wv.reshape(HKV, HD, D)
    wo_r = wo.reshape(D, H, HD)

    if not causal:
        # (128, NLT//2, NCH, 1024): pair jp holds j-tiles 2jp | 2jp+1
        mt = mask2d.T.reshape(NLT, 128, NCH, 512)
        maskg_h = np.ascontiguousarray(
            np.concatenate([mt[0::2], mt[1::2]], axis=3).transpose(1, 0, 2, 3)
        ).astype(np.float32)

    in_maps = []
    for core in range(NC):
        heads = [HPC * core + i for i in range(HPC)]
        g = heads[0] // (H // HKV)
        # weights: lhsT layout W^T tiles, partition-major
        wq_t = np.stack(
            [
                wq_r[n].T.astype(np.float16).reshape(NDT, 128, HD)
                for n in heads
            ],
            axis=0,
        )  # (HPC, NDT, 128p, 128m)
        wq_t = np.ascontiguousarray(wq_t.transpose(2, 0, 1, 3))  # (128, HPC, NDT, 128)
        wk_t = wk_r[g].T.astype(np.float16).reshape(NDT, 128, HD)
        wk_t = np.ascontiguousarray(wk_t.transpose(1, 0, 2))  # (128, NDT, 128)
        wv_t = wv_r[g].T.astype(np.float16).reshape(NDT, 128, HD)
        wv_t = np.ascontiguousarray(wv_t.transpose(1, 0, 2))
        wo_t = np.stack(
            [wo_r[:, n, :].T.astype(np.float16) for n in heads], axis=0
        )  # (HPC, 128, D)
        wo_t = np.ascontiguousarray(wo_t.transpose(1, 0, 2))  # (128, HPC, D)

        m = {
            "xq": np.stack([_chunk_tiles(q_hidden[n, 0]) for n in heads], axis=0),
            "xk": _chunk_tiles(k_hidden[g, 0]),
            "xv": _chunk_tiles(v_hidden[g, 0]),
            "wq": wq_t,
            "wk": wk_t,
            "wv": wv_t,
            "wo": wo_t,
            "cosq": cosq_h,
            "sinq": sinq_h,
            "misc": misc_h,
        }
        if causal:
            m["mask4"] = mask4_h
        else:
            m["maskg"] = maskg_h
        in_maps.append(m)

    nc = _get_program(causal)
    trace_env = os.environ.get("KERNEL_TRACE", "0")
    kwargs = {}
    if trace_env != "0":
        kwargs["trace"] = True
        if trace_env == "8":
            kwargs["trace_cores"] = list(range(NC))
    res = run_bass_kernel_spmd(nc, in_maps, core_ids=list(range(NC)), **kwargs)
    last_exec_time_ns = res.exec_time_ns
    last_mean_exec_time_ns = res.mean_exec_time_ns
    globals()["last_results"] = res.results
    globals()["last_in_maps"] = in_maps
    globals()["last_res"] = res

    out = np.empty((H, 1, L, D), dtype=np.float32)
    for core in range(NC):
        o = res.results[core]["out"]  # (HPC, NLT, 128, D) fp16
        for i in range(HPC):
            out[HPC * core + i, 0] = o[i].reshape(L, D).astype(np.float32)
    return out


# revision 38
# speedup vs baseline: 1.0293x; 1.0293x over previous
"""Trainium2 Bass kernel for per-head Llama GQA attention.

Model: H=16 q heads, HKV=4 kv heads, head_dim=128, L=2048, D=2048, B=1.
Per-head hidden streams and per-head outputs (no cross-head reduction), so
tensor-parallel over heads is embarrassingly parallel: core c owns q heads
{2c, 2c+1} and their kv head c//2.  No collectives.

The causal structure lets the whole kernel pipeline at 512-column (chunk)
granularity: attention chunk c needs K/V columns < (c+1)*512 only, so the
emission order is  [V,Q,K proj + rope of chunk c] -> attn(head0, c) ->
outproj(head0, c-1)  and the input streams never have to finish before
compute starts.  Hidden streams are host-packed chunk-major
([NCH, dtg, 128, 4, 512]) so chunk streaming stays DMA-contiguous.

Queue model (measured): gpsimd SWDGE ~266 GB/s, sync/scalar HWDGE ~100
GB/s each, ~358 GB/s per-core aggregate.  gpsimd carries xv + xq + the
fattest consts in consumption order; xk is split between sync (first
d-half) and scalar (second d-half); output stores rotate over all three
queues (16 MB of stores on a single 100 GB/s queue would otherwise be a
160 us serial tail).

Other notes:
  - per-chunk projections accumulate in single-bank (128,512) PSUM tiles;
    scores use (128,1024) pair tiles; PSUM = 4x small + 2x big = 8 banks.
  - RoPE in (HD, L) layout: rotate_half is a 128x128 signed permutation
    matmul; cos/sin are host-precomputed fp16 (HD, L) tables with
    1/sqrt(HD) folded into the q tables.
  - exp on ACT with a -2.0 bias (numerator and denominator scale
    together; keeps fp16 row-sum accumulation far from overflow).
  - softmax denominator: DVE accumulates exp tiles pairwise into an fp16
    (j_local, l) partial-sum tile; 4 single-column matmuls per chunk (acc
    slice stationary, ones column moving) give the (l,1) denominators in
    PSUM; the reciprocal is folded into the out-projection PSUM->SBUF
    copies as a per-partition scale.
"""

import os
import sys

sys.path.insert(0, "/opt/trn_rl_repo")

import numpy as np

import concourse.bass as bass
import concourse.tile as tile
from concourse import bacc, mybir
from concourse.bass_utils import run_bass_kernel_spmd

H, HKV, D, HD, L = 16, 4, 2048, 128, 2048
THETA = 10000.0
NC = 8
HPC = H // NC  # q heads per core (2)
NDT = D // 128  # d-tiles (16)
NDG = 4  # d-tile groups per stream transfer
NLT = L // 128  # l/j tiles (16)
NCH = L // 512  # 512-wide chunks (4)
F16 = mybir.dt.float16
F32 = mybir.dt.float32
EXP = mybir.ActivationFunctionType.Exp
CPY = mybir.ActivationFunctionType.Copy
EXP_BIAS = -2.0  # exp(s-2): cancels in softmax, keeps fp16 sums small

last_exec_time_ns = None
last_mean_exec_time_ns = None

_programs = {}


def _build_program(causal: bool) -> bass.Bass:
    # Bacc (not plain Bass): its compile() runs the wait-splitting passes
    # (generate_event_semaphores) that walrus requires — pseudo-DMA
    # instructions may carry at most one embedded sync wait.
    nc = bacc.Bacc(None, target_bir_lowering=False)

    # hidden streams chunk-major: [chunk, dtg, 128, di, 512]
    xq = nc.dram_tensor("xq", [HPC, NCH, NDG, 128, NDT // NDG, 512], F16,
                        kind="ExternalInput")
    xk = nc.dram_tensor("xk", [NCH, NDG, 128, NDT // NDG, 512], F16,
                        kind="ExternalInput")
    xv = nc.dram_tensor("xv", [NCH, NDG, 128, NDT // NDG, 512], F16,
                        kind="ExternalInput")
    wq = nc.dram_tensor("wq", [128, HPC, NDT, 128], F16, kind="ExternalInput")
    wk = nc.dram_tensor("wk", [128, NDT, 128], F16, kind="ExternalInput")
    wv = nc.dram_tensor("wv", [128, NDT, 128], F16, kind="ExternalInput")
    wo = nc.dram_tensor("wo", [128, HPC, D], F16, kind="ExternalInput")
    # only the q-scaled rope tables ship; k tables are derived on-device
    cosq = nc.dram_tensor("cosq", [128, L], F16, kind="ExternalInput")
    sinq = nc.dram_tensor("sinq", [128, L], F16, kind="ExternalInput")
    # misc fp16 constants: [:, :128] rotate-half perm (lhsT), [:, 128] ones
    # col, [:, 129] exp-bias col
    misc = nc.dram_tensor("misc", [128, 132], F16, kind="ExternalInput")
    if not causal:
        maskg = nc.dram_tensor("maskg", [128, NLT // 2, NCH, 1024], F32,
                               kind="ExternalInput")
    out = nc.dram_tensor("out", [HPC, NLT, 128, D], F16, kind="ExternalOutput")

    with tile.TileContext(nc) as tc:
        with (
            tc.tile_pool(name="const", bufs=1) as constp,
            tc.tile_pool(name="xvp", bufs=6) as xvp,
            tc.tile_pool(name="xkp", bufs=8) as xkp,
            tc.tile_pool(name="xqp", bufs=10) as xqp,
            tc.tile_pool(name="persist", bufs=1) as persist,
            tc.tile_pool(name="probs", bufs=8) as probsp,
            tc.tile_pool(name="accs", bufs=3) as accp,
            tc.tile_pool(name="small", bufs=3) as small,
            tc.tile_pool(name="att16", bufs=3) as att16p,
            tc.tile_pool(name="outs", bufs=4) as outsp,
            tc.tile_pool(name="recs", bufs=8) as recs,
            # PSUM: 8 banks = 4x (128,512) small + 2x (128,1024) big
            tc.tile_pool(name="psml", bufs=4, space="PSUM") as psml,
            tc.tile_pool(name="pbig", bufs=2, space="PSUM") as pbig,
        ):
            # ---- const tiles ----
            wv_sb = constp.tile([128, NDT, 128], F16, tag="wv")
            wk_sb = constp.tile([128, NDT, 128], F16, tag="wk")
            w_all = constp.tile([128, HPC, NDT, 128], F16, tag="wq")
            wo_sb = constp.tile([128, HPC, D], F16, tag="wo")
            misc_sb = constp.tile([128, 132], F16, tag="misc")
            cosk_sb = constp.tile([128, L], F16, tag="cosk")
            sink_sb = constp.tile([128, L], F16, tag="sink")
            cosq_sb = constp.tile([128, L], F16, tag="cosq")
            sinq_sb = constp.tile([128, L], F16, tag="sinq")
            identity = constp.tile([128, 128], F16, tag="ident")
            from concourse import masks as _masks

            _masks.make_identity(nc, identity[:])
            perm = misc_sb[:, 0:128]
            ones_col = misc_sb[:, 128:129]
            ebias_col = misc_sb[:, 129:130]

            # const DMAs, interleaved with the streams by deadline:
            # gpsimd: wv, wq, cosq/sinq, mask (fast queue, early deadlines)
            # sync:   misc, cosk, sink  (ahead of its xk share)
            # scalar: wk
            nc.sync.dma_start(out=misc_sb[:], in_=misc[:])
            nc.scalar.dma_start(out=wk_sb[:], in_=wk[:])

            # persistent per-core activations
            krot = persist.tile([128, L], F16, tag="krot")
            v16 = persist.tile([128, L], F16, tag="v16")
            qrot = [
                persist.tile([128, L], F16, tag=f"qrot{i}", name=f"qrot{i}")
                for i in range(HPC)
            ]

            store_rr = [0]

            def emit_store(dst, src):
                # rotate output stores g, s, a, g: gpsimd takes half
                r = store_rr[0] % 4
                store_rr[0] += 1
                eng = (nc.gpsimd, nc.sync, nc.scalar, nc.gpsimd)[r]
                eng.dma_start(out=dst, in_=src)

            # ---- chunk-granular stream loads.  One (128, 4, 512) 512 KB
            # transfer per (chunk, d-group); slab [:, di, :] is the (128 d,
            # 512 l) tile of d-tile dt = 4*g + di. ----
            def load_xv(c):
                # chunk 0 rides the HWDGE queues for d-groups 0/1 (the
                # SWDGE queue's first transfer lands ~13us late); each wv
                # quarter is interleaved ahead of its xv transfer
                ts = []
                for g in range(NDG):
                    if c == 0:
                        eng = (nc.sync, nc.scalar, nc.gpsimd, nc.gpsimd)[g]
                        eng.dma_start(
                            out=wv_sb[:, g * 4 : (g + 1) * 4, :],
                            in_=wv[:, g * 4 : (g + 1) * 4, :],
                        )
                    else:
                        eng = nc.gpsimd
                    t = xvp.tile([128, NDT // NDG, 512], F16, tag="xv",
                                 name=f"xv{c}_{g}")
                    eng.dma_start(out=t[:], in_=xv[c, g])
                    ts.append(t)
                return ts

            def load_xk(c):
                # first d-half on sync, second on scalar
                ts = []
                for g in range(NDG):
                    t = xkp.tile([128, NDT // NDG, 512], F16, tag="xk",
                                 name=f"xk{c}_{g}")
                    eng = nc.sync if g < 2 else nc.scalar
                    eng.dma_start(out=t[:], in_=xk[c, g])
                    ts.append(t)
                return ts

            def load_xq(i, c):
                ts = []
                for g in range(NDG):
                    t = xqp.tile([128, NDT // NDG, 512], F16, tag="xq",
                                 name=f"xq{i}_{c}_{g}")
                    nc.gpsimd.dma_start(out=t[:], in_=xq[i, c, g])
                    ts.append(t)
                return ts

            def proj_chunk(w_sb, x_groups, name):
                # (hd, l) projection of one 512-chunk into PSUM + fp16 cast
                pp = psml.tile([128, 512], F32, tag="ps", name=f"pp_{name}")
                for dt in range(NDT):
                    g, di = dt // NDG, dt % NDG
                    nc.tensor.matmul(
                        pp[:],
                        w_sb[:, dt, :],
                        x_groups[g][:, di, :],
                        start=(dt == 0),
                        stop=(dt == NDT - 1),
                    )
                u16 = small.tile([128, 512], F16, tag="u16", bufs=4)
                nc.scalar.copy(out=u16[:], in_=pp[:])
                return u16

            def v_fin(c, u16v):
                # 128x128 PE transposes into the (l_local, hd) blocks
                ptv = psml.tile([128, 512], F16, tag="ps", name=f"ptv{c}")
                for m in range(4):
                    nc.tensor.transpose(
                        ptv[:, m * 128 : (m + 1) * 128],
                        u16v[:, m * 128 : (m + 1) * 128],
                        identity[:],
                    )
                nc.vector.tensor_copy(
                    out=v16[:, c * 512 : (c + 1) * 512], in_=ptv[:]
                )

            def rope_fin(c, u16, cos_sb, sin_sb, dst):
                csl = slice(c * 512, (c + 1) * 512)
                rh = psml.tile([128, 512], F32, tag="ps", name=f"rh{c}")
                nc.tensor.matmul(rh[:], perm, u16[:])
                tmp = small.tile([128, 512], F16, tag="ropetmp")
                nc.vector.tensor_mul(out=tmp[:], in0=u16[:], in1=cos_sb[:, csl])
                nc.vector.tensor_mul(out=dst[:, csl], in0=rh[:], in1=sin_sb[:, csl])
                nc.vector.tensor_add(out=dst[:, csl], in0=dst[:, csl], in1=tmp[:])

            def kq_chunk(c, w_sb, x_groups, cos_sb, sin_sb, dst):
                rope_fin(c, proj_chunk(w_sb, x_groups, "kq"), cos_sb, sin_sb, dst)

            # ---- attention chunk (same dataflow as before) ----
            def attn_phase(i, c):
                njt = 4 * c + 4 if causal else NLT
                pattn = psml.tile([128, 512], F32, tag="ps", name="pattn")
                acc = accp.tile([128, 512], F16, tag="acc")
                qsl = qrot[i][:, c * 512 : (c + 1) * 512]
                for jp in range(njt // 2):
                    jt0 = 2 * jp
                    # diagonal pairs: j-tile 4c+r only attends to l >= 128r;
                    # narrow the matmuls/exp to the valid column range and
                    # zero the 128-wide boundary triangle on gpsimd
                    diag = causal and jt0 >= 4 * c
                    r0 = jt0 - 4 * c if diag else 0
                    s0, s1 = (128 * r0, 128 * r0 + 128) if diag else (0, 0)
                    sp = pbig.tile([128, 1024], F32, tag="pbig")
                    nc.tensor.matmul(
                        sp[:, s0:512],
                        krot[:, jt0 * 128 : (jt0 + 1) * 128],
                        qsl[:, s0:512],
                    )
                    nc.tensor.matmul(
                        sp[:, 512 + s1 : 1024],
                        krot[:, (jt0 + 1) * 128 : (jt0 + 2) * 128],
                        qsl[:, s1:512],
                    )
                    if not causal:
                        mg = small.tile([128, 1024], F32, tag="maskg")
                        nc.gpsimd.dma_start(out=mg[:], in_=maskg[:, jp, c, :])
                        nc.vector.tensor_add(out=sp[:], in0=sp[:], in1=mg[:])
                    pe = probsp.tile([128, 1024], F16, tag="probs")
                    nc.scalar.activation(
                        out=pe[:, s0:1024], in_=sp[:, s0:1024], func=EXP,
                        bias=ebias_col,
                    )
                    if diag:
                        # keep pe[j, l] only where l - j >= 0 in the two
                        # boundary triangles
                        for col in (s0, 512 + s1):
                            nc.gpsimd.affine_select(
                                out=pe[:, col : col + 128],
                                in_=pe[:, col : col + 128],
                                pattern=[[1, 128]],
                                compare_op=mybir.AluOpType.is_ge,
                                fill=0.0,
                                base=0,
                                channel_multiplier=-1,
                            )
                    last = jp == njt // 2 - 1
                    nc.tensor.matmul(
                        pattn[:, s0:512],
                        v16[:, jt0 * 128 : (jt0 + 1) * 128],
                        pe[:, s0:512],
                        start=(jp == 0), stop=False,
                        skip_group_check=True,
                    )
                    nc.tensor.matmul(
                        pattn[:, s1:512],
                        v16[:, (jt0 + 1) * 128 : (jt0 + 2) * 128],
                        pe[:, 512 + s1 : 1024],
                        start=False, stop=last,
                        skip_group_check=True,
                    )
                    # fp16 row-sum partials on DVE
                    if jp == 0:
                        if diag:
                            # chunk 0: first pair is diagonal (r0=0, r1=1)
                            nc.vector.tensor_copy(
                                out=acc[:, 0:128], in_=pe[:, 0:128]
                            )
                            nc.vector.tensor_add(
                                out=acc[:, 128:512],
                                in0=pe[:, 128:512],
                                in1=pe[:, 512 + s1 : 1024],
                            )
                        else:
                            nc.vector.tensor_add(
                                out=acc[:], in0=pe[:, 0:512], in1=pe[:, 512:1024]
                            )
                    else:
                        nc.vector.tensor_add(
                            out=acc[:, s0:512], in0=acc[:, s0:512],
                            in1=pe[:, s0:512],
                        )
                        nc.vector.tensor_add(
                            out=acc[:, s1:512], in0=acc[:, s1:512],
                            in1=pe[:, 512 + s1 : 1024],
                        )
                attn16 = att16p.tile([128, 512], F16, tag="attn16")
                nc.scalar.copy(out=attn16[:], in_=pattn[:])
                return acc, attn16

            def outproj_phase(i, c, acc, attn16):
                # denominators: acc slice stationary, ones column moving ->
                # (l_local, 1) column sums directly in PSUM
                pdg = psml.tile([128, 4], F32, tag="ps", name="pdg")
                for ls in range(4):
                    nc.tensor.matmul(
                        pdg[:, ls : ls + 1],
                        acc[:, ls * 128 : (ls + 1) * 128],
                        ones_col,
                    )
                recip = recs.tile([128, 4], F32, tag="recip")
                nc.vector.reciprocal(out=recip[:], in_=pdg[:])
                for ls in range(4):
                    lt = 4 * c + ls
                    a_sl = attn16[:, ls * 128 : (ls + 1) * 128]
                    r_sl = recip[:, ls : ls + 1]
                    ost = outsp.tile([128, D], F16, tag="ost")
                    for dp in range(4):
                        po = psml.tile([128, 512], F32, tag="ps", name="po")
                        nc.tensor.matmul(
                            po[:],
                            a_sl,
                            wo_sb[:, i, dp * 512 : (dp + 1) * 512],
                        )
                        hsl = slice(dp * 512, (dp + 1) * 512)
                        if dp % 2 == 0:
                            nc.vector.tensor_scalar_mul(
                                out=ost[:, hsl], in0=po[:], scalar1=r_sl
                            )
                        else:
                            nc.scalar.activation(
                                out=ost[:, hsl], in_=po[:], func=CPY, scale=r_sl
                            )
                    emit_store(out[i, lt], ost[:])

            # ================= emission =================
            def derive_k_tables():
                # k tables = q tables * sqrt(HD) (undo the folded 1/sqrt(HD))
                s = float(np.sqrt(HD))
                nc.vector.tensor_scalar_mul(
                    out=cosk_sb[:], in0=cosq_sb[:], scalar1=s
                )
                nc.vector.tensor_scalar_mul(
                    out=sink_sb[:], in0=sinq_sb[:], scalar1=s
                )

            if causal:
                xq1_tiles = [None] * NCH
                pending = None
                for c in range(NCH):
                    xv_groups = load_xv(c)
                    if c == 0:
                        # sync: misc, wv_g0, xv_c0_g0, cosq, sinq, xka...
                        # scalar: wk, wv_g1, xv_c0_g1, wq(head0), xkb...
                        nc.sync.dma_start(out=cosq_sb[:], in_=cosq[:])
                        nc.sync.dma_start(out=sinq_sb[:], in_=sinq[:])
                        derive_k_tables()
                        nc.scalar.dma_start(out=w_all[:, 0], in_=wq[:, 0])
                    xq_groups = load_xq(0, c)
                    xk_groups = load_xk(c)
                    if c == 1:
                        nc.gpsimd.dma_start(out=wo_sb[:], in_=wo[:])
                    if c >= 2:
                        if c == 2:
                            nc.gpsimd.dma_start(out=w_all[:, 1], in_=wq[:, 1])
                        xq1_tiles[c - 2] = load_xq(1, c - 2)

                    # stagger PE work so it never waits on the ACT psum->fp16
                    # casts: V proj, Q proj, V transposes, K proj, ropes
                    u16v = proj_chunk(wv_sb, xv_groups, f"v{c}")
                    u16q = proj_chunk(w_all[:, 0], xq_groups, f"q0_{c}")
                    v_fin(c, u16v)
                    u16k = proj_chunk(wk_sb, xk_groups, f"k{c}")
                    rope_fin(c, u16q, cosq_sb, sinq_sb, qrot[0])
                    rope_fin(c, u16k, cosk_sb, sink_sb, krot)
                    cur = attn_phase(0, c)
                    if pending is not None:
                        outproj_phase(*pending)
                    pending = (0, c, *cur)
                # head 1, software-pipelined one chunk deeper: the q
                # projection of chunk c+1 is emitted before attention of
                # chunk c, so PE never waits on the rope DVE chain
                xq1_tiles[2] = load_xq(1, 2)
                kq_chunk(0, w_all[:, 1], xq1_tiles[0], cosq_sb, sinq_sb, qrot[1])
                for c in range(NCH):
                    if c + 3 < NCH:
                        xq1_tiles[c + 3] = load_xq(1, c + 3)
                    if c + 1 < NCH:
                        kq_chunk(c + 1, w_all[:, 1], xq1_tiles[c + 1],
                                 cosq_sb, sinq_sb, qrot[1])
                    cur = attn_phase(1, c)
                    outproj_phase(*pending)
                    pending = (1, c, *cur)
                outproj_phase(*pending)
            else:
                # generic mask: attention chunk c needs all K/V chunks, so
                # run all projections first, then the attention loop
                nc.gpsimd.dma_start(out=w_all[:], in_=wq[:])
                nc.sync.dma_start(out=cosq_sb[:], in_=cosq[:])
                nc.sync.dma_start(out=sinq_sb[:], in_=sinq[:])
                derive_k_tables()
                nc.gpsimd.dma_start(out=wo_sb[:], in_=wo[:])
                for c in range(NCH):
                    xv_groups = load_xv(c)
                    xk_groups = load_xk(c)
                    u16v = proj_chunk(wv_sb, xv_groups, f"v{c}")
                    v_fin(c, u16v)
                    kq_chunk(c, wk_sb, xk_groups, cosk_sb, sink_sb, krot)
                for i in range(HPC):
                    for c in range(NCH):
                        xq_groups = load_xq(i, c)
                        kq_chunk(c, w_all[:, i], xq_groups, cosq_sb, sinq_sb,
                                 qrot[i])
                pending = None
                for i in range(HPC):
                    for c in range(NCH):
                        cur = attn_phase(i, c)
                        if pending is not None:
                            outproj_phase(*pending)
                        pending = (i, c, *cur)
                outproj_phase(*pending)
    nc.compile()
    return nc


def _get_program(causal: bool) -> bass.Bass:
    if causal not in _programs:
        _programs[causal] = _build_program(causal)
    return _programs[causal]


def _rope_tables(position_ids: np.ndarray):
    pos = position_ids.reshape(-1).astype(np.float32)  # (L,)
    inv_freq = (
        1.0 / (THETA ** (np.arange(0, HD, 2, dtype=np.float32) / HD))
    ).astype(np.float32)
    freqs = pos[:, None] * inv_freq[None, :]  # (L, HD/2)
    emb = np.concatenate([freqs, freqs], axis=1)  # (L, HD)
    cos = np.cos(emb).T.astype(np.float32).copy()  # (HD, L)
    sin = np.sin(emb).T.astype(np.float32).copy()
    return cos, sin


def _chunk_tiles(x):
    """(L, D) fp32 -> [NCH, NDG, 128, NDT//NDG, 512] fp16: transfer (c, g)
    is (128, 4, 512) with partition p = d-row within d-tile 4g+di, slab di
    holding l-columns [512c, 512c+512)."""
    xt = x.T.astype(np.float16).reshape(NDT, 128, L)  # (dt, p, l)
    a = xt.reshape(NDG, NDT // NDG, 128, NCH, 512)  # (g, di, p, c, l')
    return np.ascontiguousarray(a.transpose(3, 0, 2, 1, 4))


def kernel(
    q_hidden, k_hidden, v_hidden, wq, wk, wv, wo, attention_mask, position_ids
):
    global last_exec_time_ns, last_mean_exec_time_ns
    q_hidden = np.asarray(q_hidden)
    k_hidden = np.asarray(k_hidden)
    v_hidden = np.asarray(v_hidden)
    wq = np.asarray(wq, dtype=np.float32)
    wk = np.asarray(wk, dtype=np.float32)
    wv = np.asarray(wv, dtype=np.float32)
    wo = np.asarray(wo, dtype=np.float32)
    attention_mask = np.asarray(attention_mask, dtype=np.float32)
    position_ids = np.asarray(position_ids)

    mask2d = attention_mask.reshape(L, L)
    causal_ref = np.where(
        np.tril(np.ones((L, L), dtype=bool)), np.float32(0.0), np.float32(-1e9)
    )
    causal = bool(np.array_equal(mask2d, causal_ref))

    cos, sin = _rope_tables(position_ids)
    scale = np.float32(1.0 / np.sqrt(HD))
    cosq_h = (cos * scale).astype(np.float16)
    sinq_h = (sin * scale).astype(np.float16)

    misc_h = np.zeros((128, 132), dtype=np.float16)
    # rotate-half: rh = P @ q with P[i, i+64] = -1 (i<64), P[i, i-64] = +1;
    # stored as lhsT = P^T
    for a in range(64):
        misc_h[a, a + 64] = np.float16(1.0)  # P^T[a, a+64] = P[a+64, a] = +1
        misc_h[a + 64, a] = np.float16(-1.0)  # P^T[a+64, a] = P[a, a+64] = -1
    misc_h[:, 128] = np.float16(1.0)  # ones column
    misc_h[:, 129] = np.float16(EXP_BIAS)  # exp bias column

    wq_r = wq.reshape(H, HD, D)
    wk_r = wk.reshape(HKV, HD, D)
    wv_r = wv.reshape(HKV, HD, D)
    wo_r = wo.reshape(D, H, HD)

    if not causal:
        # (128, NLT//2, NCH, 1024): pair jp holds j-tiles 2jp | 2jp+1
        mt = mask2d.T.reshape(NLT, 128, NCH, 512)
        maskg_h = np.ascontiguousarray(
            np.concatenate([mt[0::2], mt[1::2]], axis=3).transpose(1, 0, 2, 3)
        ).astype(np.float32)

    in_maps = []
    for core in range(NC):
        heads = [HPC * core + i for i in range(HPC)]
        g = heads[0] // (H // HKV)
        # weights: lhsT layout W^T tiles, partition-major
        wq_t = np.stack(
            [
                wq_r[n].T.astype(np.float16).reshape(NDT, 128, HD)
                for n in heads
            ],
            axis=0,
        )  # (HPC, NDT, 128p, 128m)
        wq_t = np.ascontiguousarray(wq_t.transpose(2, 0, 1, 3))  # (128, HPC, NDT, 128)
        wk_t = wk_r[g].T.astype(np.float16).reshape(NDT, 128, HD)
        wk_t = np.ascontiguousarray(wk_t.transpose(1, 0, 2))  # (128, NDT, 128)
        wv_t = wv_r[g].T.astype(np.float16).reshape(NDT, 128, HD)
        wv_t = np.ascontiguousarray(wv_t.transpose(1, 0, 2))
        wo_t = np.stack(
            [wo_r[:, n, :].T.astype(np.float16) for n in heads], axis=0
        )  # (HPC, 128, D)
        wo_t = np.ascontiguousarray(wo_t.transpose(1, 0, 2))  # (128, HPC, D)

        m = {
            "xq": np.stack([_chunk_tiles(q_hidden[n, 0]) for n in heads], axis=0),
            "xk": _chunk_tiles(k_hidden[g, 0]),
            "xv": _chunk_tiles(v_hidden[g, 0]),
            "wq": wq_t,
            "wk": wk_t,
            "wv": wv_t,
            "wo": wo_t,
            "cosq": cosq_h,
            "sinq": sinq_h,
            "misc": misc_h,
        }
        if not causal:
            m["maskg"] = maskg_h
        in_maps.append(m)

    nc = _get_program(causal)
    trace_env = os.environ.get("KERNEL_TRACE", "0")
    kwargs = {}
    if trace_env != "0":
        kwargs["trace"] = True
        if trace_env == "8":
            kwargs["trace_cores"] = list(range(NC))
    res = run_bass_kernel_spmd(nc, in_maps, core_ids=list(range(NC)), **kwargs)
    last_exec_time_ns = res.exec_time_ns
    last_mean_exec_time_ns = res.mean_exec_time_ns
    globals()["last_results"] = res.results
    globals()["last_in_maps"] = in_maps
    globals()["last_res"] = res

    out = np.empty((H, 1, L, D), dtype=np.float32)
    for core in range(NC):
        o = res.results[core]["out"]  # (HPC, NLT, 128, D) fp16
        for i in range(HPC):
            out[HPC * core + i, 0] = o[i].reshape(L, D).astype(np.float32)
    return out


# revision 43
# speedup vs baseline: 1.0353x; 1.0058x over previous
"""Trainium2 Bass kernel for per-head Llama GQA attention.

Model: H=16 q heads, HKV=4 kv heads, head_dim=128, L=2048, D=2048, B=1.
Per-head hidden streams and per-head outputs (no cross-head reduction), so
tensor-parallel over heads is embarrassingly parallel: core c owns q heads
{2c, 2c+1} and their kv head c//2.  No collectives.

The causal structure lets the whole kernel pipeline at 512-column (chunk)
granularity: attention chunk c needs K/V columns < (c+1)*512 only, so the
emission order is  [V,Q,K proj + rope of chunk c] -> attn(head0, c) ->
outproj(head0, c-1)  and the input streams never have to finish before
compute starts.  Hidden streams are host-packed chunk-major
([NCH, dtg, 128, 4, 512]) so chunk streaming stays DMA-contiguous.

Queue model (measured): gpsimd SWDGE ~266 GB/s, sync/scalar HWDGE ~100
GB/s each, ~358 GB/s per-core aggregate.  gpsimd carries xv + xq + the
fattest consts in consumption order; xk is split between sync (first
d-half) and scalar (second d-half); output stores rotate over all three
queues (16 MB of stores on a single 100 GB/s queue would otherwise be a
160 us serial tail).

Other notes:
  - per-chunk projections accumulate in single-bank (128,512) PSUM tiles;
    scores use (128,1024) pair tiles; PSUM = 4x small + 2x big = 8 banks.
  - RoPE in (HD, L) layout: rotate_half is a 128x128 signed permutation
    matmul; cos/sin are host-precomputed fp16 (HD, L) tables with
    1/sqrt(HD) folded into the q tables.
  - exp on ACT with a -2.0 bias (numerator and denominator scale
    together; keeps fp16 row-sum accumulation far from overflow).
  - softmax denominator: DVE accumulates exp tiles pairwise into an fp16
    (j_local, l) partial-sum tile; 4 single-column matmuls per chunk (acc
    slice stationary, ones column moving) give the (l,1) denominators in
    PSUM; the reciprocal is folded into the out-projection PSUM->SBUF
    copies as a per-partition scale.
"""

import os
import sys

sys.path.insert(0, "/opt/trn_rl_repo")

import numpy as np

import concourse.bass as bass
import concourse.tile as tile
from concourse import bacc, mybir
from concourse.bass_utils import run_bass_kernel_spmd

H, HKV, D, HD, L = 16, 4, 2048, 128, 2048
THETA = 10000.0
NC = 8
HPC = H // NC  # q heads per core (2)
NDT = D // 128  # d-tiles (16)
NDG = 4  # d-tile groups per stream transfer
NLT = L // 128  # l/j tiles (16)
NCH = L // 512  # 512-wide chunks (4)
F16 = mybir.dt.float16
F32 = mybir.dt.float32
EXP = mybir.ActivationFunctionType.Exp
CPY = mybir.ActivationFunctionType.Copy
EXP_BIAS = -2.0  # exp(s-2): cancels in softmax, keeps fp16 sums small

last_exec_time_ns = None
last_mean_exec_time_ns = None

_programs = {}


def _build_program(causal: bool) -> bass.Bass:
    # Bacc (not plain Bass): its compile() runs the wait-splitting passes
    # (generate_event_semaphores) that walrus requires — pseudo-DMA
    # instructions may carry at most one embedded sync wait.
    nc = bacc.Bacc(None, target_bir_lowering=False)

    # hidden streams chunk-major: [chunk, dtg, 128, di, 512]
    xq = nc.dram_tensor("xq", [HPC, NCH, NDG, 128, NDT // NDG, 512], F16,
                        kind="ExternalInput")
    xk = nc.dram_tensor("xk", [NCH, NDG, 128, NDT // NDG, 512], F16,
                        kind="ExternalInput")
    xv = nc.dram_tensor("xv", [NCH, NDG, 128, NDT // NDG, 512], F16,
                        kind="ExternalInput")
    wq = nc.dram_tensor("wq", [128, HPC, NDT, 128], F16, kind="ExternalInput")
    wk = nc.dram_tensor("wk", [128, NDT, 128], F16, kind="ExternalInput")
    wv = nc.dram_tensor("wv", [128, NDT, 128], F16, kind="ExternalInput")
    wo = nc.dram_tensor("wo", [128, HPC, D], F16, kind="ExternalInput")
    # only the q-scaled rope tables ship; k tables are derived on-device
    cosq = nc.dram_tensor("cosq", [128, L], F16, kind="ExternalInput")
    sinq = nc.dram_tensor("sinq", [128, L], F16, kind="ExternalInput")
    # misc fp16 constants: [:, :128] rotate-half perm (lhsT), [:, 128] ones
    # col, [:, 129] exp-bias col
    misc = nc.dram_tensor("misc", [128, 132], F16, kind="ExternalInput")
    if not causal:
        maskg = nc.dram_tensor("maskg", [128, NLT // 2, NCH, 1024], F32,
                               kind="ExternalInput")
    out = nc.dram_tensor("out", [HPC, NLT, 128, D], F16, kind="ExternalOutput")

    with tile.TileContext(nc) as tc:
        with (
            tc.tile_pool(name="const", bufs=1) as constp,
            tc.tile_pool(name="xvp", bufs=6) as xvp,
            tc.tile_pool(name="xkp", bufs=8) as xkp,
            tc.tile_pool(name="xqp", bufs=10) as xqp,
            tc.tile_pool(name="persist", bufs=1) as persist,
            tc.tile_pool(name="probs", bufs=8) as probsp,
            tc.tile_pool(name="accs", bufs=3) as accp,
            tc.tile_pool(name="small", bufs=3) as small,
            tc.tile_pool(name="att16", bufs=3) as att16p,
            tc.tile_pool(name="outs", bufs=4) as outsp,
            tc.tile_pool(name="recs", bufs=8) as recs,
            # PSUM: 8 banks = 4x (128,512) small + 2x (128,1024) big
            tc.tile_pool(name="psml", bufs=4, space="PSUM") as psml,
            tc.tile_pool(name="pbig", bufs=2, space="PSUM") as pbig,
        ):
            # ---- const tiles ----
            wv_sb = constp.tile([128, NDT, 128], F16, tag="wv")
            wk_sb = constp.tile([128, NDT, 128], F16, tag="wk")
            w_all = constp.tile([128, HPC, NDT, 128], F16, tag="wq")
            wo_sb = constp.tile([128, HPC, D], F16, tag="wo")
            misc_sb = constp.tile([128, 132], F16, tag="misc")
            cosk_sb = constp.tile([128, L], F16, tag="cosk")
            sink_sb = constp.tile([128, L], F16, tag="sink")
            cosq_sb = constp.tile([128, L], F16, tag="cosq")
            sinq_sb = constp.tile([128, L], F16, tag="sinq")
            identity = constp.tile([128, 128], F16, tag="ident")
            from concourse import masks as _masks

            _masks.make_identity(nc, identity[:])
            perm = misc_sb[:, 0:128]
            ones_col = misc_sb[:, 128:129]
            ebias_col = misc_sb[:, 129:130]

            # persistent per-core activations
            krot = persist.tile([128, L], F16, tag="krot")
            v16 = persist.tile([128, L], F16, tag="v16")
            qrot = [
                persist.tile([128, L], F16, tag=f"qrot{i}", name=f"qrot{i}")
                for i in range(HPC)
            ]

            store_rr = [0]

            def emit_store(dst, src):
                # rotate output stores g, s, a, g: gpsimd takes half
                r = store_rr[0] % 4
                store_rr[0] += 1
                eng = (nc.gpsimd, nc.sync, nc.scalar, nc.gpsimd)[r]
                eng.dma_start(out=dst, in_=src)

            # ---- chunk-granular stream loads.  One (128, 4, 512) 512 KB
            # transfer per (chunk, d-group); slab [:, di, :] is the (128 d,
            # 512 l) tile of d-tile dt = 4*g + di. ----
            def load_xv(c):
                ts = []
                for g in range(NDG):
                    t = xvp.tile([128, NDT // NDG, 512], F16, tag="xv",
                                 name=f"xv{c}_{g}")
                    nc.gpsimd.dma_start(out=t[:], in_=xv[c, g])
                    ts.append(t)
                return ts

            def load_first_group():
                """Chunk-0 loads, d-tile-granular on the HWDGE queues (the
                SWDGE queue's first transfer lands ~13us after launch, the
                HWDGE queues ~8us).  sync/scalar feed V d-tiles 0..7 paired
                with their wv slices; gpsimd catches up with the rest."""
                xv_ts, xq_ts, xk_ts = [], [], []
                for g, eng in ((0, nc.sync), (1, nc.scalar)):
                    t = xvp.tile([128, NDT // NDG, 512], F16, tag="xv",
                                 name=f"xv0_{g}")
                    for di in range(NDT // NDG):
                        dt = NDG * g + di
                        eng.dma_start(
                            out=wv_sb[:, dt, :], in_=wv[:, dt, :]
                        )
                        eng.dma_start(out=t[:, di], in_=xv[0, g, :, di])
                    xv_ts.append(t)
                for g in (2, 3):
                    nc.gpsimd.dma_start(
                        out=wv_sb[:, g * 4 : (g + 1) * 4, :],
                        in_=wv[:, g * 4 : (g + 1) * 4, :],
                    )
                    t = xvp.tile([128, NDT // NDG, 512], F16, tag="xv",
                                 name=f"xv0_{g}")
                    nc.gpsimd.dma_start(out=t[:], in_=xv[0, g])
                    xv_ts.append(t)
                # q hidden: d-groups 0/1 on sync, 2/3 on gpsimd; head-0
                # weights on scalar ahead of its wk
                nc.scalar.dma_start(out=w_all[:, 0], in_=wq[:, 0])
                for g in range(NDG):
                    t = xqp.tile([128, NDT // NDG, 512], F16, tag="xq",
                                 name=f"xq0_0_{g}")
                    (nc.sync if g < 2 else nc.gpsimd).dma_start(
                        out=t[:], in_=xq[0, 0, g]
                    )
                    xq_ts.append(t)
                nc.scalar.dma_start(out=wk_sb[:], in_=wk[:])
                for g in range(NDG):
                    t = xkp.tile([128, NDT // NDG, 512], F16, tag="xk",
                                 name=f"xk0_{g}")
                    nc.gpsimd.dma_start(out=t[:], in_=xk[0, g])
                    xk_ts.append(t)
                nc.sync.dma_start(out=misc_sb[:], in_=misc[:])
                nc.sync.dma_start(out=cosq_sb[:], in_=cosq[:])
                nc.sync.dma_start(out=sinq_sb[:], in_=sinq[:])
                derive_k_tables()
                return xv_ts, xq_ts, xk_ts

            def load_xk(c):
                # first d-half on sync, second on scalar
                ts = []
                for g in range(NDG):
                    t = xkp.tile([128, NDT // NDG, 512], F16, tag="xk",
                                 name=f"xk{c}_{g}")
                    eng = nc.sync if g < 2 else nc.scalar
                    eng.dma_start(out=t[:], in_=xk[c, g])
                    ts.append(t)
                return ts

            def load_xq(i, c):
                ts = []
                for g in range(NDG):
                    t = xqp.tile([128, NDT // NDG, 512], F16, tag="xq",
                                 name=f"xq{i}_{c}_{g}")
                    nc.gpsimd.dma_start(out=t[:], in_=xq[i, c, g])
                    ts.append(t)
                return ts

            def proj_chunk(w_sb, x_groups, name):
                # (hd, l) projection of one 512-chunk into PSUM + fp16 cast
                pp = psml.tile([128, 512], F32, tag="ps", name=f"pp_{name}")
                for dt in range(NDT):
                    g, di = dt // NDG, dt % NDG
                    nc.tensor.matmul(
                        pp[:],
                        w_sb[:, dt, :],
                        x_groups[g][:, di, :],
                        start=(dt == 0),
                        stop=(dt == NDT - 1),
                    )
                u16 = small.tile([128, 512], F16, tag="u16", bufs=4)
                nc.scalar.copy(out=u16[:], in_=pp[:])
                return u16

            def v_fin(c, u16v):
                # 128x128 PE transposes into the (l_local, hd) blocks
                ptv = psml.tile([128, 512], F16, tag="ps", name=f"ptv{c}")
                for m in range(4):
                    nc.tensor.transpose(
                        ptv[:, m * 128 : (m + 1) * 128],
                        u16v[:, m * 128 : (m + 1) * 128],
                        identity[:],
                    )
                nc.vector.tensor_copy(
                    out=v16[:, c * 512 : (c + 1) * 512], in_=ptv[:]
                )

            def rope_fin(c, u16, cos_sb, sin_sb, dst):
                csl = slice(c * 512, (c + 1) * 512)
                rh = psml.tile([128, 512], F32, tag="ps", name=f"rh{c}")
                nc.tensor.matmul(rh[:], perm, u16[:])
                tmp = small.tile([128, 512], F16, tag="ropetmp")
                nc.vector.tensor_mul(out=tmp[:], in0=u16[:], in1=cos_sb[:, csl])
                nc.vector.tensor_mul(out=dst[:, csl], in0=rh[:], in1=sin_sb[:, csl])
                nc.vector.tensor_add(out=dst[:, csl], in0=dst[:, csl], in1=tmp[:])

            def kq_chunk(c, w_sb, x_groups, cos_sb, sin_sb, dst):
                rope_fin(c, proj_chunk(w_sb, x_groups, "kq"), cos_sb, sin_sb, dst)

            # ---- attention chunk (same dataflow as before) ----
            def attn_phase(i, c):
                njt = 4 * c + 4 if causal else NLT
                pattn = psml.tile([128, 512], F32, tag="ps", name="pattn")
                acc = accp.tile([128, 512], F16, tag="acc")
                qsl = qrot[i][:, c * 512 : (c + 1) * 512]
                for jp in range(njt // 2):
                    jt0 = 2 * jp
                    # diagonal pairs: j-tile 4c+r only attends to l >= 128r;
                    # narrow the matmuls/exp to the valid column range and
                    # zero the 128-wide boundary triangle on gpsimd
                    diag = causal and jt0 >= 4 * c
                    r0 = jt0 - 4 * c if diag else 0
                    s0, s1 = (128 * r0, 128 * r0 + 128) if diag else (0, 0)
                    sp = pbig.tile([128, 1024], F32, tag="pbig")
                    nc.tensor.matmul(
                        sp[:, s0:512],
                        krot[:, jt0 * 128 : (jt0 + 1) * 128],
                        qsl[:, s0:512],
                    )
                    nc.tensor.matmul(
                        sp[:, 512 + s1 : 1024],
                        krot[:, (jt0 + 1) * 128 : (jt0 + 2) * 128],
                        qsl[:, s1:512],
                    )
                    if not causal:
                        mg = small.tile([128, 1024], F32, tag="maskg")
                        nc.gpsimd.dma_start(out=mg[:], in_=maskg[:, jp, c, :])
                        nc.vector.tensor_add(out=sp[:], in0=sp[:], in1=mg[:])
                    pe = probsp.tile([128, 1024], F16, tag="probs")
                    # exp per 512-half: the first attn matmul starts while
                    # the second half's exp still runs
                    nc.scalar.activation(
                        out=pe[:, s0:512], in_=sp[:, s0:512], func=EXP,
                        bias=ebias_col,
                    )
                    nc.scalar.activation(
                        out=pe[:, 512 + s1 : 1024], in_=sp[:, 512 + s1 : 1024],
                        func=EXP, bias=ebias_col,
                    )
                    if diag:
                        # keep pe[j, l] only where l - j >= 0 in the two
                        # boundary triangles
                        for col in (s0, 512 + s1):
                            nc.gpsimd.affine_select(
                                out=pe[:, col : col + 128],
                                in_=pe[:, col : col + 128],
                                pattern=[[1, 128]],
                                compare_op=mybir.AluOpType.is_ge,
                                fill=0.0,
                                base=0,
                                channel_multiplier=-1,
                            )
                    last = jp == njt // 2 - 1
                    nc.tensor.matmul(
                        pattn[:, s0:512],
                        v16[:, jt0 * 128 : (jt0 + 1) * 128],
                        pe[:, s0:512],
                        start=(jp == 0), stop=False,
                        skip_group_check=True,
                    )
                    nc.tensor.matmul(
                        pattn[:, s1:512],
                        v16[:, (jt0 + 1) * 128 : (jt0 + 2) * 128],
                        pe[:, 512 + s1 : 1024],
                        start=False, stop=last,
                        skip_group_check=True,
                    )
                    # fp16 row-sum partials on DVE
                    if jp == 0:
                        if diag:
                            # chunk 0: first pair is diagonal (r0=0, r1=1)
                            nc.vector.tensor_copy(
                                out=acc[:, 0:128], in_=pe[:, 0:128]
                            )
                            nc.vector.tensor_add(
                                out=acc[:, 128:512],
                                in0=pe[:, 128:512],
                                in1=pe[:, 512 + s1 : 1024],
                            )
                        else:
                            nc.vector.tensor_add(
                                out=acc[:], in0=pe[:, 0:512], in1=pe[:, 512:1024]
                            )
                    else:
                        nc.vector.tensor_add(
                            out=acc[:, s0:512], in0=acc[:, s0:512],
                            in1=pe[:, s0:512],
                        )
                        nc.vector.tensor_add(
                            out=acc[:, s1:512], in0=acc[:, s1:512],
                            in1=pe[:, 512 + s1 : 1024],
                        )
                attn16 = att16p.tile([128, 512], F16, tag="attn16")
                nc.scalar.copy(out=attn16[:], in_=pattn[:])
                return acc, attn16

            def outproj_phase(i, c, acc, attn16):
                # denominators: acc slice stationary, ones column moving ->
                # (l_local, 1) column sums directly in PSUM
                pdg = psml.tile([128, 4], F32, tag="ps", name="pdg")
                for ls in range(4):
                    nc.tensor.matmul(
                        pdg[:, ls : ls + 1],
                        acc[:, ls * 128 : (ls + 1) * 128],
                        ones_col,
                    )
                recip = recs.tile([128, 4], F32, tag="recip")
                nc.vector.reciprocal(out=recip[:], in_=pdg[:])
                for ls in range(4):
                    lt = 4 * c + ls
                    a_sl = attn16[:, ls * 128 : (ls + 1) * 128]
                    r_sl = recip[:, ls : ls + 1]
                    ost = outsp.tile([128, D], F16, tag="ost")
                    for dp in range(4):
                        po = psml.tile([128, 512], F32, tag="ps", name="po")
                        nc.tensor.matmul(
                            po[:],
                            a_sl,
                            wo_sb[:, i, dp * 512 : (dp + 1) * 512],
                        )
                        hsl = slice(dp * 512, (dp + 1) * 512)
                        if dp % 2 == 0:
                            nc.vector.tensor_scalar_mul(
                                out=ost[:, hsl], in0=po[:], scalar1=r_sl
                            )
                        else:
                            nc.scalar.activation(
                                out=ost[:, hsl], in_=po[:], func=CPY, scale=r_sl
                            )
                    emit_store(out[i, lt], ost[:])

            # ================= emission =================
            def derive_k_tables():
                # k tables = q tables * sqrt(HD) (undo the folded 1/sqrt(HD))
                s = float(np.sqrt(HD))
                nc.vector.tensor_scalar_mul(
                    out=cosk_sb[:], in0=cosq_sb[:], scalar1=s
                )
                nc.vector.tensor_scalar_mul(
                    out=sink_sb[:], in0=sinq_sb[:], scalar1=s
                )

            if causal:
                xq1_tiles = [None] * NCH
                pending = None
                for c in range(NCH):
                    if c == 0:
                        xv_groups, xq_groups, xk_groups = load_first_group()
                    else:
                        xv_groups = load_xv(c)
                        xq_groups = load_xq(0, c)
                        xk_groups = load_xk(c)
                    if c == 1:
                        nc.gpsimd.dma_start(out=wo_sb[:], in_=wo[:])
                    if c >= 2:
                        if c == 2:
                            nc.gpsimd.dma_start(out=w_all[:, 1], in_=wq[:, 1])
                        xq1_tiles[c - 2] = load_xq(1, c - 2)

                    # stagger PE work so it never waits on the ACT psum->fp16
                    # casts: V proj, Q proj, V transposes, K proj, ropes
                    u16v = proj_chunk(wv_sb, xv_groups, f"v{c}")
                    u16q = proj_chunk(w_all[:, 0], xq_groups, f"q0_{c}")
                    v_fin(c, u16v)
                    u16k = proj_chunk(wk_sb, xk_groups, f"k{c}")
                    rope_fin(c, u16q, cosq_sb, sinq_sb, qrot[0])
                    rope_fin(c, u16k, cosk_sb, sink_sb, krot)
                    cur = attn_phase(0, c)
                    if pending is not None:
                        outproj_phase(*pending)
                    pending = (0, c, *cur)
                # head 1, software-pipelined one chunk deeper: the q
                # projection of chunk c+1 is emitted before attention of
                # chunk c, so PE never waits on the rope DVE chain
                xq1_tiles[2] = load_xq(1, 2)
                kq_chunk(0, w_all[:, 1], xq1_tiles[0], cosq_sb, sinq_sb, qrot[1])
                for c in range(NCH):
                    if c + 3 < NCH:
                        xq1_tiles[c + 3] = load_xq(1, c + 3)
                    if c + 1 < NCH:
                        kq_chunk(c + 1, w_all[:, 1], xq1_tiles[c + 1],
                                 cosq_sb, sinq_sb, qrot[1])
                    cur = attn_phase(1, c)
                    outproj_phase(*pending)
                    pending = (1, c, *cur)
                outproj_phase(*pending)
            else:
                # generic mask: attention chunk c needs all K/V chunks, so
                # run all projections first, then the attention loop
                nc.sync.dma_start(out=misc_sb[:], in_=misc[:])
                nc.scalar.dma_start(out=wk_sb[:], in_=wk[:])
                nc.gpsimd.dma_start(out=wv_sb[:], in_=wv[:])
                nc.gpsimd.dma_start(out=w_all[:], in_=wq[:])
                nc.sync.dma_start(out=cosq_sb[:], in_=cosq[:])
                nc.sync.dma_start(out=sinq_sb[:], in_=sinq[:])
                derive_k_tables()
                nc.gpsimd.dma_start(out=wo_sb[:], in_=wo[:])
                for c in range(NCH):
                    xv_groups = load_xv(c)
                    xk_groups = load_xk(c)
                    u16v = proj_chunk(wv_sb, xv_groups, f"v{c}")
                    v_fin(c, u16v)
                    kq_chunk(c, wk_sb, xk_groups, cosk_sb, sink_sb, krot)
                for i in range(HPC):
                    for c in range(NCH):
                        xq_groups = load_xq(i, c)
                        kq_chunk(c, w_all[:, i], xq_groups, cosq_sb, sinq_sb,
                                 qrot[i])
                pending = None
                for i in range(HPC):
                    for c in range(NCH):
                        cur = attn_phase(i, c)
                        if pending is not None:
                            outproj_phase(*pending)
                        pending = (i, c, *cur)
                outproj_phase(*pending)
    nc.compile()
    return nc


def _get_program(causal: bool) -> bass.Bass:
    if causal not in _programs:
        _programs[causal] = _build_program(causal)
    return _programs[causal]


def _rope_tables(position_ids: np.ndarray):
    pos = position_ids.reshape(-1).astype(np.float32)  # (L,)
    inv_freq = (
        1.0 / (THETA ** (np.arange(0, HD, 2, dtype=np.float32) / HD))
    ).astype(np.float32)
    freqs = pos[:, None] * inv_freq[None, :]  # (L, HD/2)
    emb = np.concatenate([freqs, freqs], axis=1)  # (L, HD)
    cos = np.cos(emb).T.astype(np.float32).copy()  # (HD, L)
    sin = np.sin(emb).T.astype(np.float32).copy()
    return cos, sin


def _chunk_tiles(x):
    """(L, D) fp32 -> [NCH, NDG, 128, NDT//NDG, 512] fp16: transfer (c, g)
    is (128, 4, 512) with partition p = d-row within d-tile 4g+di, slab di
    holding l-columns [512c, 512c+512)."""
    xt = x.T.astype(np.float16).reshape(NDT, 128, L)  # (dt, p, l)
    a = xt.reshape(NDG, NDT // NDG, 128, NCH, 512)  # (g, di, p, c, l')
    return np.ascontiguousarray(a.transpose(3, 0, 2, 1, 4))


def kernel(
    q_hidden, k_hidden, v_hidden, wq, wk, wv, wo, attention_mask, position_ids
):
    global last_exec_time_ns, last_mean_exec_time_ns
    q_hidden = np.asarray(q_hidden)
    k_hidden = np.asarray(k_hidden)
    v_hidden = np.asarray(v_hidden)
    wq = np.asarray(wq, dtype=np.float32)
    wk = np.asarray(wk, dtype=np.float32)
    wv = np.asarray(wv, dtype=np.float32)
    wo = np.asarray(wo, dtype=np.float32)
    attention_mask = np.asarray(attention_mask, dtype=np.float32)
    position_ids = np.asarray(position_ids)

    mask2d = attention_mask.reshape(L, L)
    causal_ref = np.where(
        np.tril(np.ones((L, L), dtype=bool)), np.float32(0.0), np.float32(-1e9)
    )
    causal = bool(np.array_equal(mask2d, causal_ref))

    cos, sin = _rope_tables(position_ids)
    scale = np.float32(1.0 / np.sqrt(HD))
    cosq_h = (cos * scale).astype(np.float16)
    sinq_h = (sin * scale).astype(np.float16)

    misc_h = np.zeros((128, 132), dtype=np.float16)
    # rotate-half: rh = P @ q with P[i, i+64] = -1 (i<64), P[i, i-64] = +1;
    # stored as lhsT = P^T
    for a in range(64):
        misc_h[a, a + 64] = np.float16(1.0)  # P^T[a, a+64] = P[a+64, a] = +1
        misc_h[a + 64, a] = np.float16(-1.0)  # P^T[a+64, a] = P[a, a+64] = -1
    misc_h[:, 128] = np.float16(1.0)  # ones column
    misc_h[:, 129] = np.float16(EXP_BIAS)  # exp bias column

    wq_r = wq.reshape(H, HD, D)
    wk_r = wk.reshape(HKV, HD, D)
    wv_r = wv.reshape(HKV, HD, D)
    wo_r = wo.reshape(D, H, HD)

    if not causal:
        # (128, NLT//2, NCH, 1024): pair jp holds j-tiles 2jp | 2jp+1
        mt = mask2d.T.reshape(NLT, 128, NCH, 512)
        maskg_h = np.ascontiguousarray(
            np.concatenate([mt[0::2], mt[1::2]], axis=3).transpose(1, 0, 2, 3)
        ).astype(np.float32)

    in_maps = []
    for core in range(NC):
        heads = [HPC * core + i for i in range(HPC)]
        g = heads[0] // (H // HKV)
        # weights: lhsT layout W^T tiles, partition-major
        wq_t = np.stack(
            [
                wq_r[n].T.astype(np.float16).reshape(NDT, 128, HD)
                for n in heads
            ],
            axis=0,
        )  # (HPC, NDT, 128p, 128m)
        wq_t = np.ascontiguousarray(wq_t.transpose(2, 0, 1, 3))  # (128, HPC, NDT, 128)
        wk_t = wk_r[g].T.astype(np.float16).reshape(NDT, 128, HD)
        wk_t = np.ascontiguousarray(wk_t.transpose(1, 0, 2))  # (128, NDT, 128)
        wv_t = wv_r[g].T.astype(np.float16).reshape(NDT, 128, HD)
        wv_t = np.ascontiguousarray(wv_t.transpose(1, 0, 2))
        wo_t = np.stack(
            [wo_r[:, n, :].T.astype(np.float16) for n in heads], axis=0
        )  # (HPC, 128, D)
        wo_t = np.ascontiguousarray(wo_t.transpose(1, 0, 2))  # (128, HPC, D)

        m = {
            "xq": np.stack([_chunk_tiles(q_hidden[n, 0]) for n in heads], axis=0),
            "xk": _chunk_tiles(k_hidden[g, 0]),
            "xv": _chunk_tiles(v_hidden[g, 0]),
            "wq": wq_t,
            "wk": wk_t,
            "wv": wv_t,
            "wo": wo_t,
            "cosq": cosq_h,
            "sinq": sinq_h,
            "misc": misc_h,
        }
        if not causal:
            m["maskg"] = maskg_h
        in_maps.append(m)

    nc = _get_program(causal)
    trace_env = os.environ.get("KERNEL_TRACE", "0")
    kwargs = {}
    if trace_env != "0":
        kwargs["trace"] = True
        if trace_env == "8":
            kwargs["trace_cores"] = list(range(NC))
    res = run_bass_kernel_spmd(nc, in_maps, core_ids=list(range(NC)), **kwargs)
    last_exec_time_ns = res.exec_time_ns
    last_mean_exec_time_ns = res.mean_exec_time_ns
    globals()["last_results"] = res.results
    globals()["last_in_maps"] = in_maps
    globals()["last_res"] = res

    out = np.empty((H, 1, L, D), dtype=np.float32)
    for core in range(NC):
        o = res.results[core]["out"]  # (HPC, NLT, 128, D) fp16
        for i in range(HPC):
            out[HPC * core + i, 0] = o[i].reshape(L, D).astype(np.float32)
    return out
